# revision 3
# baseline (speedup 1.0000x reference)
"""BigBird block-sparse attention (3-block sliding window, zero-padded edges)
for Trainium2, SPMD over 8 NeuronCores, data-parallel over the batch dim.

Full computation per batch element b:
  q/k/v = x @ W{q,k,v}        -> [N, H*64]
  block attention: each 128-row query block attends keys of blocks
  {i-1, i, i+1}; out-of-range blocks are zero keys/values that contribute
  exp(0)=1 to the softmax denominator only.
  y = attn_out @ Wo + bo

Matmuls run in bf16 (fp32 accumulation in PSUM).
"""

import os
import numpy as np

import concourse.bass as bass
import concourse.mybir as mybir
import concourse.tile as tile
from concourse import bacc
from concourse.bass_utils import run_bass_kernel_spmd
from concourse.masks import make_identity

B, N, DIM = 16, 1536, 1536
H, DK, DV, BS = 8, 64, 64, 128
NB = N // BS                     # 12 blocks per sequence
NCORES = 8
BPC = B // NCORES                # batch elements per core
SCALE = 1.0 / np.sqrt(DK)        # 0.125

f32 = mybir.dt.float32
bf16 = mybir.dt.bfloat16
EXP = mybir.ActivationFunctionType.Exp

_NC_CACHE = {}
LAST_RESULTS = None


def _emit(nc):
    X = nc.dram_tensor("x", [BPC, N, DIM], f32, kind="ExternalInput")
    WQ = nc.dram_tensor("Wq", [DIM, H * DK], f32, kind="ExternalInput")
    WK = nc.dram_tensor("Wk", [DIM, H * DK], f32, kind="ExternalInput")
    WV = nc.dram_tensor("Wv", [DIM, H * DV], f32, kind="ExternalInput")
    WO = nc.dram_tensor("Wo", [H * DV, DIM], f32, kind="ExternalInput")
    BO = nc.dram_tensor("bo", [DIM], f32, kind="ExternalInput")
    Y = nc.dram_tensor("y", [BPC, N, DIM], f32, kind="ExternalOutput")

    KC = DIM // 128              # 12 contraction chunks for projections
    HV = H * DV                  # 512

    with tile.TileContext(nc) as tc:
        with (
            tc.tile_pool(name="wts", bufs=1) as wts,
            tc.tile_pool(name="stage", bufs=3) as stage,
            tc.tile_pool(name="xtp", bufs=2) as xtp,
            tc.tile_pool(name="qkv", bufs=1) as qkv,
            tc.tile_pool(name="expp", bufs=4) as expp,
            tc.tile_pool(name="otp", bufs=13) as otp,
            tc.tile_pool(name="rcp", bufs=4) as rcp,
            tc.tile_pool(name="bcp", bufs=4) as bcp,
            tc.tile_pool(name="yp", bufs=2) as yp,
            tc.tile_pool(name="bigp", bufs=4, space="PSUM") as bigp,
            tc.tile_pool(name="scp", bufs=2, space="PSUM") as scp,
            tc.tile_pool(name="pop", bufs=2, space="PSUM") as pop,
        ):
            # ---- constants ----
            ident = wts.tile([128, 128], f32)
            make_identity(nc, ident)
            ones_row = wts.tile([1, DV], bf16)
            nc.vector.memset(ones_row, 1.0)
            pad128 = wts.tile([1, 1], f32)
            nc.vector.memset(pad128, 128.0)
            bo_bc = wts.tile([128, DIM], f32)
            bo_ap = BO[:]
            nc.sync.dma_start(
                out=bo_bc,
                in_=bass.AP(tensor=bo_ap.tensor, offset=bo_ap.offset,
                            ap=[[0, 128]] + list(bo_ap.ap)),
            )

            # ---- weights: DMA f32 -> SBUF, cast to bf16 ----
            wq_bf = wts.tile([128, KC, HV], bf16)
            wk_bf = wts.tile([128, KC, HV], bf16)
            wv_bf = wts.tile([128, KC, HV], bf16)
            wo_bf = wts.tile([128, HV // 128, DIM], bf16)
            for (wdram, wsb) in ((WQ, wq_bf), (WK, wk_bf), (WV, wv_bf)):
                wr = wdram.rearrange("(n p) m -> p n m", p=128)   # [128, 12, 512]
                for g in range(4):
                    st = stage.tile([128, 3, HV], f32, name=f"wst{g}", tag="stage")
                    nc.sync.dma_start(out=st, in_=wr[:, 3 * g:3 * (g + 1), :])
                    nc.vector.tensor_copy(wsb[:, 3 * g:3 * (g + 1), :], st)
            wor = WO.rearrange("(n p) m -> p n m", p=128)          # [128, 4, 1536]
            for c in range(HV // 128):
                st = stage.tile([128, DIM], f32, name=f"wost{c}", tag="stage")
                nc.sync.dma_start(out=st, in_=wor[:, c, :])
                nc.vector.tensor_copy(wo_bf[:, c, :], st)

            for b in range(BPC):
                # per-batch-element activation buffers (transposed layouts)
                qT_buf = qkv.tile([128, 4, N], bf16, name=f"qT{b}", tag="qT")
                kT_buf = qkv.tile([128, 4, N], bf16, name=f"kT{b}", tag="kT")
                v_buf = qkv.tile([128, NB, H, DV + 1], bf16, name=f"v{b}", tag="v")
                nc.vector.memset(v_buf[:, :, :, DV:DV + 1], 1.0)

                # ---- Phase P: projections per 128-row block ----
                for i in range(NB):
                    xst = stage.tile([128, DIM], f32, name=f"xst{b}_{i}", tag="stage")
                    nc.sync.dma_start(out=xst, in_=X[b, i * BS:(i + 1) * BS, :])
                    xT = xtp.tile([128, KC, BS], bf16, name=f"xT{b}_{i}", tag="xT")
                    for kc in range(KC):
                        pt = bigp.tile([128, BS], f32, name=f"pt{b}_{i}_{kc}", tag="big")
                        nc.tensor.transpose(pt, xst[:, kc * 128:(kc + 1) * 128], ident)
                        nc.vector.tensor_copy(xT[:, kc, :], pt)

                    pq = bigp.tile([128, HV], f32, name=f"pq{b}_{i}", tag="big")
                    for c in range(4):
                        for kc in range(KC):
                            nc.tensor.matmul(
                                pq[:, c * 128:(c + 1) * 128],
                                wq_bf[:, kc, c * 128:(c + 1) * 128],
                                xT[:, kc, :],
                                start=(c == 0 and kc == 0),
                                stop=(c == 3 and kc == KC - 1),
                            )
                    nc.vector.tensor_copy(
                        qT_buf[:, :, i * BS:(i + 1) * BS],
                        pq.rearrange("p (c r) -> p c r", c=4),
                    )

                    pk = bigp.tile([128, HV], f32, name=f"pk{b}_{i}", tag="big")
                    for c in range(4):
                        for kc in range(KC):
                            nc.tensor.matmul(
                                pk[:, c * 128:(c + 1) * 128],
                                wk_bf[:, kc, c * 128:(c + 1) * 128],
                                xT[:, kc, :],
                                start=(c == 0 and kc == 0),
                                stop=(c == 3 and kc == KC - 1),
                            )
                    nc.vector.tensor_copy(
                        kT_buf[:, :, i * BS:(i + 1) * BS],
                        pk.rearrange("p (c r) -> p c r", c=4),
                    )

                    pv = bigp.tile([128, HV], f32, name=f"pv{b}_{i}", tag="big")
                    for kc in range(KC):
                        nc.tensor.matmul(pv, xT[:, kc, :], wv_bf[:, kc, :],
                                         start=(kc == 0), stop=(kc == KC - 1))
                    nc.vector.tensor_copy(
                        v_buf[:, i, :, 0:DV],
                        pv.rearrange("p (h d) -> p h d", h=H),
                    )

                # ---- Phase A: attention, h outer ----
                oT = [otp.tile([128, 4, BS], bf16, name=f"oT{b}_{i}", tag="oT")
                      for i in range(NB)]

                for h in range(H):
                    pb, ch = (h % 2) * DK, h // 2
                    expt = [None] * NB

                    def do_outT(i, h=h, pb=pb, ch=ch, expt=None, oT=oT, b=b):
                        po = pop.tile([128, 2, BS], f32,
                                      name=f"po{b}_{h}_{i}", tag="po")
                        js = [j for j in (i - 1, i, i + 1) if 0 <= j < NB]
                        for idx, j in enumerate(js):
                            col = (i - max(j - 1, 0)) * BS
                            nc.tensor.matmul(
                                po[0:DV + 1, 0, :],
                                v_buf[:, j, h, :],
                                expt[j][:, col:col + BS],
                                start=(idx == 0),
                                stop=(idx == len(js) - 1),
                            )
                        if i == 0 or i == NB - 1:
                            # zero-padded edge block: 128 keys with logit 0
                            nc.scalar.activation(
                                out=po[DV:DV + 1, 0, :], in_=po[DV:DV + 1, 0, :],
                                func=mybir.ActivationFunctionType.Identity,
                                bias=pad128, scale=1.0)
                        rc = rcp.tile([1, BS], bf16, name=f"rc{b}_{h}_{i}", tag="rc")
                        with nc.allow_low_precision("softmax recip bf16"):
                            nc.vector.reciprocal(rc, po[DV:DV + 1, 0, :])
                        nc.tensor.matmul(po[0:DV, 1, :], ones_row, rc,
                                         start=True, stop=True)
                        bc = bcp.tile([DV, BS], f32, name=f"bc{b}_{h}_{i}", tag="bc")
                        nc.vector.tensor_copy(bc, po[0:DV, 1, :])
                        nc.vector.tensor_mul(oT[i][pb:pb + DV, ch, :],
                                             po[0:DV, 0, :], bc)

                    for j in range(NB):
                        qlo, qhi = max(j - 1, 0), min(j + 1, NB - 1)
                        nq = (qhi - qlo + 1) * BS
                        psc = scp.tile([128, 3 * BS], f32,
                                       name=f"psc{b}_{h}_{j}", tag="sc")
                        nc.tensor.matmul(
                            psc[:, 0:nq],
                            kT_buf[pb:pb + DK, ch, j * BS:(j + 1) * BS],
                            qT_buf[pb:pb + DK, ch, qlo * BS:(qhi + 1) * BS],
                            start=True, stop=True,
                        )
                        et = expp.tile([128, 3 * BS], bf16,
                                       name=f"et{b}_{h}_{j}", tag="exp")
                        nc.scalar.activation(out=et[:, 0:nq], in_=psc[:, 0:nq],
                                             func=EXP, scale=float(SCALE))
                        expt[j] = et
                        if j >= 1:
                            do_outT(j - 1, expt=expt)
                    do_outT(NB - 1, expt=expt)

                # ---- Phase O: output projection ----
                for i in range(NB):
                    ysb = yp.tile([128, DIM], f32, name=f"y{b}_{i}", tag="y")
                    for n in range(3):
                        py = bigp.tile([128, 512], f32,
                                       name=f"py{b}_{i}_{n}", tag="big")
                        for c in range(4):
                            nc.tensor.matmul(py, oT[i][:, c, :],
                                             wo_bf[:, c, n * 512:(n + 1) * 512],
                                             start=(c == 0), stop=(c == 3))
                        nc.vector.tensor_add(ysb[:, n * 512:(n + 1) * 512], py,
                                             bo_bc[:, n * 512:(n + 1) * 512])
                    nc.sync.dma_start(out=Y[b, i * BS:(i + 1) * BS, :], in_=ysb)


def _get_nc():
    if "nc" not in _NC_CACHE:
        nc = bacc.Bacc("TRN2", target_bir_lowering=False, debug=False)
        _emit(nc)
        nc.finalize()
        _NC_CACHE["nc"] = nc
    return _NC_CACHE["nc"]


def kernel(x, Wq, Wk, Wv, Wo, bo):
    global LAST_RESULTS
    x = np.ascontiguousarray(np.asarray(x, dtype=np.float32))
    Wq = np.ascontiguousarray(np.asarray(Wq, dtype=np.float32))
    Wk = np.ascontiguousarray(np.asarray(Wk, dtype=np.float32))
    Wv = np.ascontiguousarray(np.asarray(Wv, dtype=np.float32))
    Wo = np.ascontiguousarray(np.asarray(Wo, dtype=np.float32))
    bo = np.ascontiguousarray(np.asarray(bo, dtype=np.float32))

    nc = _get_nc()
    in_maps = [
        {"x": x[c * BPC:(c + 1) * BPC], "Wq": Wq, "Wk": Wk, "Wv": Wv,
         "Wo": Wo, "bo": bo}
        for c in range(NCORES)
    ]
    trace = bool(int(os.environ.get("KERNEL_TRACE", "0")))
    res = run_bass_kernel_spmd(nc, in_maps, list(range(NCORES)), trace=trace)
    LAST_RESULTS = res
    return np.concatenate([res.results[c]["y"] for c in range(NCORES)], axis=0)
